# revision 41
# baseline (speedup 1.0000x reference)
"""Multi-head attention (B=4, S=2048, D=1024, H=16) on 8 trn2 NeuronCores.

Sharding: core c -> (batch b = c//2, head-group hg = c%2 of 8 heads).
Each core computes q/k/v projections for its 8 heads, attention, and a
partial output projection. Host sums the two partials per batch + b_O.

Design: the ACT engine's exp stream is the roofline (~1.15us per
[128,1024] score tile); everything else is scheduled to hide under it.
  - ACT runs ONLY the 256 exp instructions (scale=1/8, bias=-4:
    softmax is shift-invariant; keeps exp small and overflow-safe).
  - v-hat is [128, head, po, 65] bf16 with a ones column so softmax Z
    falls out of the PV matmul (psum row 64). PV emission is delayed
    two slots so block boundaries never stall on stage/recip drains.
  - scores matmuls: head pair at base partitions 0/64 -> row-tile
    concurrency on the PE array (tile_position auto-derived).
  - loops are qc-outer / hp-inner so the output projection can
    PSUM-accumulate across all 4 head pairs per piece; pieces fire in
    the next qc block's kt slots (weighted pacing).
  - recipZ broadcast runs on the idle Pool engine (partition_broadcast)
  - projection bias epilogues run on DVE (tensor_scalar_add), not ACT.
  - projections are JIT slot-planned pieces inside the attention
    stream: the serial prefix is only kT[0] + qT[0]/qc0 + v half.
    Remaining kT[hp]/qT[hp] chunks fire just before the block that
    reads them; q half1 fires paced during qc1; v st8-15 inside qc0/hp0.
  - qc3's output projection switches to per-hp SBUF accumulation so
    pieces drain during the last blocks instead of a serial tail.
"""
import sys
from collections import deque

if '/opt/trn_rl_repo' not in sys.path:
    sys.path.insert(0, '/opt/trn_rl_repo')

import ml_dtypes
import numpy as np

import concourse.bass as bass
import concourse.tile as tile
from concourse import bacc, mybir, library_config
from concourse.bass_utils import run_bass_kernel_spmd

N_CORES = 8
B, S, D = 4, 2048, 1024
H = 16
DH = 64                 # head dim
HC = 8                  # heads per core
C = HC * DH             # per-core projection width = 512
F32 = mybir.dt.float32
BF16 = mybir.dt.bfloat16
FP8 = mybir.dt.float8e4

NKT = S // 128          # 16 s-tiles of 128
NJ = NKT // 2           # 8 kt-pairs
NM = C // 128           # 4 head pairs
NDK = D // 128          # 8 contraction tiles for projections
SCALE = 1.0 / np.sqrt(DH)
EXP_BIAS = -4.0         # logit shift: exp(s/8 - 4), cancels in softmax
                        # (max scaled logit measured 8.29; e^4.3=73 < fp8 240)
VPAD = 80 if False else 65   # pad only needed for fp8 DoubleRow stride

PROJ_DT = BF16          # projection inputs (XT, W)
QK_DT = BF16            # qT/kT tiles (scores matmul inputs)
OUT_DT = BF16           # attn_outT + Wo (output projection inputs)
PV_FP8 = False          # fp8e4+DoubleRow PV: ~3.3e-2 err, too coarse


def prep(x, dt):
    return np.ascontiguousarray(x).astype(ml_dtypes.bfloat16)


def build():
    nc = bacc.Bacc("TRN2", target_bir_lowering=False, debug=False,
                   num_devices=N_CORES)
    XqT = nc.dram_tensor("XqT", [D, S], PROJ_DT, kind="ExternalInput").ap()
    XkT = nc.dram_tensor("XkT", [D, S], PROJ_DT, kind="ExternalInput").ap()
    XvT = nc.dram_tensor("XvT", [D, S], PROJ_DT, kind="ExternalInput").ap()
    Wq = nc.dram_tensor("Wq", [D, C], PROJ_DT, kind="ExternalInput").ap()
    Wk = nc.dram_tensor("Wk", [D, C], PROJ_DT, kind="ExternalInput").ap()
    Wv = nc.dram_tensor("Wv", [D, C], PROJ_DT, kind="ExternalInput").ap()
    Wo = nc.dram_tensor("Wo", [C, D], OUT_DT, kind="ExternalInput").ap()
    bq = nc.dram_tensor("bq", [C], F32, kind="ExternalInput").ap()
    bk = nc.dram_tensor("bk", [C], F32, kind="ExternalInput").ap()
    bv = nc.dram_tensor("bv", [C], F32, kind="ExternalInput").ap()
    OP = nc.dram_tensor("OP", [S, D], F32, kind="ExternalOutput").ap()

    with tile.TileContext(nc) as tc:
        _build_body(nc, tc, XqT, XkT, XvT, Wq, Wk, Wv, Wo, bq, bk, bv, OP)
    nc.compile()
    return nc


def _build_body(nc, tc, XqT, XkT, XvT, Wq, Wk, Wv, Wo, bq, bk, bv, OP):
    from contextlib import ExitStack
    with ExitStack() as stack:
        ep = stack.enter_context
        consts = ep(tc.tile_pool(name="consts", bufs=1))
        qkp = ep(tc.tile_pool(name="qk", bufs=2 * NM))
        vhp = ep(tc.tile_pool(name="vh", bufs=NJ))
        aop = ep(tc.tile_pool(name="aout", bufs=NM))
        wop = ep(tc.tile_pool(name="wo", bufs=NM))
        # 5 big X tiles alive (xk0,xk1,xq0,xv0,xv1); xq1 reuses the
        # xk0 buffer only after all kT chunks are emitted (qc0 end)
        xtp = ep(tc.tile_pool(name="xt", bufs=5))
        wp = ep(tc.tile_pool(name="w", bufs=3))
        ptp = ep(tc.tile_pool(name="pt", bufs=3))
        stg = ep(tc.tile_pool(name="stg", bufs=2))
        nrm = ep(tc.tile_pool(name="nrm", bufs=6))
        osg = ep(tc.tile_pool(name="osg", bufs=2))
        sp = ep(tc.tile_pool(name="sps", bufs=2, space="PSUM"))
        pvp = ep(tc.tile_pool(name="pv", bufs=2, space="PSUM"))
        scr = ep(tc.tile_pool(name="scr", bufs=2, space="PSUM"))

        nc.gpsimd.load_library(library_config.attn)

        # ---- constants ----
        neg2 = consts.tile([128, 1], F32)
        nc.vector.memset(neg2, EXP_BIAS)
        bias_t = consts.tile([128, 2 * NM], F32)
        for i, b_ in enumerate((bq, bk)):
            nc.gpsimd.dma_start(
                out=bias_t[:, i * NM:(i + 1) * NM],
                in_=b_.rearrange("(m p) -> p m", p=128))
        bvb = consts.tile([128, C], F32)
        nc.gpsimd.dma_start(
            out=bvb,
            in_=bass.AP(tensor=bv.tensor, offset=0, ap=[[0, 128], [1, C]]))

        wo_tiles = []
        for m in range(NM):
            w = wop.tile([128, D], OUT_DT, tag="wo", name=f"wo{m}")
            nc.gpsimd.dma_start(out=w, in_=Wo[m * 128:(m + 1) * 128, :])
            wo_tiles.append(w)

        # ---- persistent big tiles ----
        qT = [qkp.tile([128, S], QK_DT, tag="qk", name=f"qT{m}")
              for m in range(NM)]
        kT = [qkp.tile([128, S], QK_DT, tag="qk", name=f"kT{m}")
              for m in range(NM)]
        attn_outT = [aop.tile([128, S], OUT_DT, tag="aout", name=f"aoT{m}")
                     for m in range(NM)]
        PT_DT = FP8 if PV_FP8 else BF16
        # vh2[j]: [128(k within tile), head, po(kt pair sub), VPAD]
        vh2 = [vhp.tile([128, HC, 2, VPAD], PT_DT, tag="vh", name=f"vh{j}")
               for j in range(NJ)]
        for j in range(NJ):
            nc.vector.memset(vh2[j][:, :, :, DH:DH + 1], 1.0)

        # ---- projection helpers: single-DMA big tiles ----
        # X half: [128, dk, 1024] one DMA; W: [128, dk, C] one DMA
        def load_x(XT, half, tag):
            xt = xtp.tile([128, NDK, S // 2], PROJ_DT, tag="xt",
                          name=f"x{tag}{half}")
            nc.sync.dma_start(
                out=xt,
                in_=XT[:, half * (S // 2):(half + 1) * (S // 2)].rearrange(
                    "(a p) s -> p a s", p=128))
            return xt

        def load_w(W, tag):
            w = wp.tile([128, NDK, C], PROJ_DT, tag="w", name=f"w{tag}")
            nc.sync.dma_start(
                out=w, in_=W.rearrange("(a p) c -> p a c", p=128))
            return w

        def load_w_m(W, tag, w=None, ms=range(NM)):
            # per-m column slices: the first chunk needs only m=0
            if w is None:
                w = wp.tile([128, NDK, C], PROJ_DT, tag="w", name=f"w{tag}")
            for m in ms:
                nc.sync.dma_start(
                    out=w[:, :, m * 128:(m + 1) * 128],
                    in_=W[:, m * 128:(m + 1) * 128].rearrange(
                        "(a p) c -> p a c", p=128))
            return w

        def qk_mms(ps, xt, w, m, sc, dks):
            for dk in dks:
                nc.tensor.matmul(
                    ps,
                    w[:, dk, m * 128:(m + 1) * 128],
                    xt[:, dk, sc * 512:(sc + 1) * 512],
                    start=(dk == 0), stop=(dk == NDK - 1))

        def qk_epi(ps, m, half, sc, bcol, outs):
            s0 = half * (S // 2) + sc * 512
            with nc.allow_low_precision(reason="proj epilogue"):
                nc.vector.tensor_scalar_add(
                    outs[m][:, s0:s0 + 512], ps,
                    bias_t[:, bcol + m:bcol + m + 1])

        def qk_piece(xt, w, m, half, sc, bcol, outs):
            """One [128,512] projection chunk: 8 acc-mm + DVE epilogue."""
            ps = scr.tile([128, 512], F32, tag="scr")
            qk_mms(ps, xt, w, m, sc, range(NDK))
            qk_epi(ps, m, half, sc, bcol, outs)

        def qk_split(xt, w, m, half, sc, bcol, outs):
            """Two half-pieces (accumulation group pauses mid-chain)."""
            box = {}

            def p1():
                ps = scr.tile([128, 512], F32, tag="scr")
                box['ps'] = ps
                qk_mms(ps, xt, w, m, sc, range(4))

            def p2():
                qk_mms(box['ps'], xt, w, m, sc, range(4, NDK))
                qk_epi(box['ps'], m, half, sc, bcol, outs)

            return p1, p2

        def v_piece(xt, w, st):
            """v-hat for s-tile st -> vh2[st//2][:, :, st%2, 0:64] (+bias)."""
            ps = scr.tile([128, C], F32, tag="scr")
            for dk in range(NDK):
                nc.tensor.matmul(
                    ps,
                    xt[:, dk, (st % 8) * 128:(st % 8 + 1) * 128],
                    w[:, dk, :],
                    start=(dk == 0), stop=(dk == NDK - 1))
            dst = vh2[st // 2][:, :, st % 2, 0:DH]
            with nc.allow_low_precision(reason="v epilogue"):
                nc.vector.tensor_add(
                    dst,
                    ps.rearrange("p (h d) -> p h d", h=HC),
                    bvb.rearrange("p (h d) -> p h d", h=HC))

        o3 = {}

        def o3_acc(i):
            # accumulators live in dead X-tile buffers (xv0/xq0 slots)
            if 'a' not in o3:
                t1 = xtp.tile([128, 4, 512], F32, tag="xt", name="o3a")
                t2 = xtp.tile([128, 4, 512], F32, tag="xt", name="o3b")
                o3['a'] = (t1, t2)
            t = o3['a'][i // 4]
            return t[:, i % 4, :]

        def outproj3_piece(st, oc, hp):
            """qc3: per-hp mm + SBUF accumulate; DMA on the last hp."""
            ps = scr.tile([128, 512], F32, tag="scr")
            nc.tensor.matmul(
                ps,
                attn_outT[hp][:, st * 128:(st + 1) * 128],
                wo_tiles[hp][:, oc * 512:(oc + 1) * 512],
                start=True, stop=True)
            oa = o3_acc((st - 12) * 2 + oc)
            if hp == 0:
                nc.vector.tensor_copy(oa, ps)
            else:
                nc.vector.tensor_add(oa, oa, ps)
            if hp == NM - 1:
                nc.sync.dma_start(
                    out=OP[st * 128:(st + 1) * 128,
                           oc * 512:(oc + 1) * 512],
                    in_=oa)

        def outproj_piece(st, oc):
            """OP[st,oc] chunk: 4 hp-accumulated mm + stage + DMA."""
            ps = scr.tile([128, 512], F32, tag="scr")
            for hp in range(NM):
                nc.tensor.matmul(
                    ps,
                    attn_outT[hp][:, st * 128:(st + 1) * 128],
                    wo_tiles[hp][:, oc * 512:(oc + 1) * 512],
                    start=(hp == 0), stop=(hp == NM - 1))
            oa = osg.tile([128, 512], F32, tag="osg")
            nc.vector.tensor_copy(oa, ps)
            nc.sync.dma_start(
                out=OP[st * 128:(st + 1) * 128, oc * 512:(oc + 1) * 512],
                in_=oa)

        # ---------------- prefix (serial head, ACT idle) ----------------
        # DMA order matches the PE's FIFO consumption order exactly:
        # kchunk(0,h0,*) -> qchunk(0) -> v0-5 -> kT half1 + v6-15 in (0,0)
        xk = [load_x(XkT, 0, "k"), None]
        wk = load_w_m(Wk, "k", ms=[0, 1])
        xv = [load_x(XvT, 0, "v"), None]
        wv = load_w(Wv, "v")
        xq = [load_x(XqT, 0, "q"), None]
        wq = load_w_m(Wq, "q", ms=[0, 1])
        xk[1] = load_x(XkT, 1, "k")
        xv[1] = load_x(XvT, 1, "v")
        load_w_m(Wk, "k", w=wk, ms=[2, 3])
        load_w_m(Wq, "q", w=wq, ms=[2, 3])

        def kchunk(m, half, sc):
            qk_piece(xk[half], wk, m, half, sc, NM, kT)

        def kchunk_split(m, half, sc):
            return qk_split(xk[half], wk, m, half, sc, NM, kT)

        def qchunk(m, half, sc):
            qk_piece(xq[half], wq, m, half, sc, 0, qT)

        def qchunk_split(m, half, sc):
            return qk_split(xq[half], wq, m, half, sc, 0, qT)

        # minimal prefix in PE-FIFO = DMA-arrival order: kT[0] half0
        # (+ kT[1]'s first chunk in the v-DMA wait window), v0-5, then
        # qT[0]/qT[1] qc0 -- PV(0) has v0 ready when exp starts
        kchunk(0, 0, 0)
        kchunk(0, 0, 1)
        kchunk(1, 0, 0)
        for st in range(6):
            v_piece(xv[0], wv, st)
        qchunk(0, 0, 0)
        qchunk(1, 0, 0)

        # ---------------- attention ----------------
        # slot_plan: mandatory pieces at exact kt slots (deps: a chunk
        # must be emitted before the instruction that reads it).
        # deferred: paced deque of (weight, fn) for slack work.
        deferred = deque()
        state = {'cooldown': 0}

        def fire():
            if state['cooldown'] > 0:
                state['cooldown'] -= 1
                return
            if deferred:
                w, fn = deferred.popleft()
                fn()
                state['cooldown'] = w - 1

        def plan_for(qc, hp):
            # a chunk must be EMITTED before the instruction reading it;
            # late slots of the previous block prefetch the next block.
            # post-plan pieces fire between exp and PV (so scores/exp of
            # the slot are not queued behind them on the PE).
            plan = {}
            post = {}

            def put(kt, fn):
                plan.setdefault(kt, []).append(fn)

            if qc == 0 and hp == 0:
                # v6-15: a slot ahead of the PV that reads them
                for st in range(6, NKT):
                    post.setdefault(st - 1, []).append(
                        lambda st=st: v_piece(xv[st // 8], wv, st))
                put(3, lambda: kchunk(0, 1, 0))
                put(4, lambda: kchunk(0, 1, 1))
            if qc == 0 and hp in (1, 2):
                # prefetch next hp's first chunks
                put(13, lambda hp=hp: kchunk(hp + 1, 0, 0))
                put(15, lambda hp=hp: qchunk(hp + 1, 0, 0))
            if qc == 0 and hp > 0:
                for s0, half, sc in ((2, 0, 1), (6, 1, 0), (10, 1, 1)):
                    p1, p2 = kchunk_split(hp, half, sc)
                    put(s0, p1)
                    put(s0 + 1, p2)
            if qc == 0 and hp == 3:
                p1, p2 = qchunk_split(0, 0, 1)
                put(13, p1)
                put(15, p2)
            if qc == 1:
                # prefetch next block's qc1 q chunk, split across slots
                if hp < 3:
                    p1, p2 = qchunk_split(hp + 1, 0, 1)
                    put(13, p1)
                    put(15, p2)
            return plan, post

        for qc in range(4):
            q0 = qc * 512
            for hp in range(NM):
                slot_plan, slot_post = plan_for(qc, hp)
                pvA = pvp.tile([DH + 1, 512], F32, tag="pv",
                               name=f"pvA{qc}_{hp}")
                pvB = pvp.tile([DH + 1, 512], F32, tag="pv",
                               name=f"pvB{qc}_{hp}")
                pend_pv = deque()
                for kt in range(NKT):
                    for fn in slot_plan.get(kt, ()):
                        fn()
                    if kt >= 2:
                        # keep block-start slots clear: the first scores
                        # must not queue behind deferred PE pieces
                        fire()
                    sps = sp.tile([128, 1024], F32, tag="sps")
                    for hh in range(2):
                        dlo = hh * DH
                        nc.tensor.matmul(
                            sps[:, hh * 512:(hh + 1) * 512],
                            kT[hp][dlo:dlo + DH, kt * 128:(kt + 1) * 128],
                            qT[hp][dlo:dlo + DH, q0:q0 + 512],
                            start=True, stop=True)
                    pt_k = ptp.tile([128, 2, 512], PT_DT, tag="pt")
                    with nc.allow_low_precision(reason="exp out"):
                        nc.scalar.activation(
                            out=pt_k.rearrange("p a b -> p (a b)"),
                            in_=sps,
                            func=mybir.ActivationFunctionType.Exp,
                            bias=neg2, scale=float(SCALE))
                    for fn in slot_post.get(kt, ()):
                        fn()
                    # PV(kt) emits two slots later: the first PVs of a
                    # block then don't wait on the previous block's
                    # stage/recip draining the pv psum buffers
                    if len(pend_pv) >= 2:
                        pend_pv.popleft()()

                    def mk_pv(kt=kt, pt_k=pt_k):
                        j, po = kt // 2, kt % 2
                        nc.tensor.matmul(
                            pvA, vh2[j][:, 2 * hp, po, 0:DH + 1],
                            pt_k[:, 0, :],
                            start=(kt == 0), stop=(kt == NKT - 1))
                        nc.tensor.matmul(
                            pvB, vh2[j][:, 2 * hp + 1, po, 0:DH + 1],
                            pt_k[:, 1, :],
                            start=(kt == 0), stop=(kt == NKT - 1))
                    pend_pv.append(mk_pv)
                while pend_pv:
                    pend_pv.popleft()()

                # ---- immediate tail: free pv psum fast ----
                stA = stg.tile([DH, 512], BF16, tag="stg")
                stB = stg.tile([DH, 512], BF16, tag="stg")
                rzA = nrm.tile([1, 512], BF16, tag="rz")
                rzB = nrm.tile([1, 512], BF16, tag="rz")
                with nc.allow_low_precision(reason="stage"):
                    nc.vector.tensor_copy(stA, pvA[0:DH, :])
                    nc.vector.tensor_copy(stB, pvB[0:DH, :])
                with nc.allow_low_precision(reason="recipZ bf16"):
                    nc.vector.reciprocal(out=rzA, in_=pvA[DH:DH + 1, :])
                    nc.vector.reciprocal(out=rzB, in_=pvB[DH:DH + 1, :])

                def mk_tail(hp=hp, q0=q0, stA=stA, stB=stB,
                            rzA=rzA, rzB=rzB):
                    bcA = nrm.tile([DH, 512], BF16, tag="bc")
                    bcB = nrm.tile([DH, 512], BF16, tag="bc")

                    def t0():
                        nc.gpsimd.partition_broadcast(bcA, rzA)

                    def t1():
                        nc.gpsimd.partition_broadcast(bcB, rzB)

                    def t2():
                        with nc.allow_low_precision(reason="attn out"):
                            nc.vector.tensor_mul(
                                attn_outT[hp][0:DH, q0:q0 + 512], stA, bcA)

                    def t3():
                        with nc.allow_low_precision(reason="attn out"):
                            nc.vector.tensor_mul(
                                attn_outT[hp][DH:128, q0:q0 + 512], stB, bcB)

                    return [(1, t0), (1, t1), (1, t2), (1, t3)]

                deferred.extend(mk_tail())

                if qc == 3:
                    deferred.extend(
                        (1, (lambda st=st, oc=oc, hp=hp:
                             outproj3_piece(st, oc, hp)))
                        for st in range(12, 16) for oc in range(2))

                if qc == 0 and hp == 3:
                    # q half1: DMA now (reuses the xk0 buffer; all kT
                    # half0 chunks already emitted). sc=0 chunks (read
                    # by qc2) fire in qc1; sc=1 (read by qc3) in qc2.
                    xq[1] = load_x(XqT, 1, "q")
                    for m in range(NM):
                        p1, p2 = qchunk_split(m, 1, 0)
                        deferred.extend([(2, p1), (2, p2)])
                if qc == 1 and hp == 3:
                    for m in range(NM):
                        p1, p2 = qchunk_split(m, 1, 1)
                        deferred.extend([(2, p1), (2, p2)])
            if qc < 3:
                # hp-accumulated outproj fires in the next qc's slots
                deferred.extend(
                    (2, (lambda st=st, oc=oc: outproj_piece(st, oc)))
                    for st in range(qc * 4, qc * 4 + 4) for oc in range(2))

        while deferred:
            deferred.popleft()[1]()


_NC_CACHE = None
_last_in_maps = None


def _get_nc():
    global _NC_CACHE
    if _NC_CACHE is None:
        _NC_CACHE = build()
    return _NC_CACHE


def kernel(Q, K, V, W_Q, b_Q, W_K, b_K, W_V, b_V, W_O, b_O):
    global _last_in_maps
    Q = np.asarray(Q, dtype=np.float32)
    K = np.asarray(K, dtype=np.float32)
    V = np.asarray(V, dtype=np.float32)
    nc = _get_nc()

    XqTs = [prep(Q[b].T, PROJ_DT) for b in range(B)]
    XkTs = [prep(K[b].T, PROJ_DT) for b in range(B)]
    XvTs = [prep(V[b].T, PROJ_DT) for b in range(B)]
    Wqs = [prep(np.asarray(W_Q)[:, hg * C:(hg + 1) * C], PROJ_DT)
           for hg in range(2)]
    Wks = [prep(np.asarray(W_K)[:, hg * C:(hg + 1) * C], PROJ_DT)
           for hg in range(2)]
    Wvs = [prep(np.asarray(W_V)[:, hg * C:(hg + 1) * C], PROJ_DT)
           for hg in range(2)]
    Wos = [prep(np.asarray(W_O)[hg * C:(hg + 1) * C, :], OUT_DT)
           for hg in range(2)]
    bqs = [np.ascontiguousarray(np.asarray(b_Q, np.float32)[hg * C:(hg + 1) * C])
           for hg in range(2)]
    bks = [np.ascontiguousarray(np.asarray(b_K, np.float32)[hg * C:(hg + 1) * C])
           for hg in range(2)]
    bvs = [np.ascontiguousarray(np.asarray(b_V, np.float32)[hg * C:(hg + 1) * C])
           for hg in range(2)]

    in_maps = []
    for c in range(N_CORES):
        b, hg = c // 2, c % 2
        in_maps.append({
            "XqT": XqTs[b], "XkT": XkTs[b], "XvT": XvTs[b],
            "Wq": Wqs[hg], "Wk": Wks[hg], "Wv": Wvs[hg], "Wo": Wos[hg],
            "bq": bqs[hg], "bk": bks[hg], "bv": bvs[hg],
        })
    _last_in_maps = in_maps
    res = run_bass_kernel_spmd(nc, in_maps, list(range(N_CORES)))
    out = np.empty((B, S, D), dtype=np.float32)
    bO = np.asarray(b_O, dtype=np.float32)
    for b in range(B):
        out[b] = res.results[2 * b]["OP"] + res.results[2 * b + 1]["OP"] + bO
    return out


# revision 42
# speedup vs baseline: 1.0013x; 1.0013x over previous
"""Multi-head attention (B=4, S=2048, D=1024, H=16) on 8 trn2 NeuronCores.

Sharding: core c -> (batch b = c//2, head-group hg = c%2 of 8 heads).
Each core computes q/k/v projections for its 8 heads, attention, and a
partial output projection. Host sums the two partials per batch + b_O.

Design: the ACT engine's exp stream is the roofline (~1.15us per
[128,1024] score tile); everything else is scheduled to hide under it.
  - ACT runs ONLY the 256 exp instructions (scale=1/8, bias=-4:
    softmax is shift-invariant; keeps exp small and overflow-safe).
  - v-hat is [128, head, po, 65] bf16 with a ones column so softmax Z
    falls out of the PV matmul (psum row 64). PV emission is delayed
    two slots so block boundaries never stall on stage/recip drains.
  - scores matmuls: head pair at base partitions 0/64 -> row-tile
    concurrency on the PE array (tile_position auto-derived).
  - loops are qc-outer / hp-inner so the output projection can
    PSUM-accumulate across all 4 head pairs per piece; pieces fire in
    the next qc block's kt slots (weighted pacing).
  - recipZ broadcast runs on the idle Pool engine (partition_broadcast)
  - projection bias epilogues run on DVE (tensor_scalar_add), not ACT.
  - projections are JIT slot-planned pieces inside the attention
    stream: the serial prefix is only kT[0] + qT[0]/qc0 + v half.
    Remaining kT[hp]/qT[hp] chunks fire just before the block that
    reads them; q half1 fires paced during qc1; v st8-15 inside qc0/hp0.
  - qc3's output projection switches to per-hp SBUF accumulation so
    pieces drain during the last blocks instead of a serial tail.
"""
import sys
from collections import deque

if '/opt/trn_rl_repo' not in sys.path:
    sys.path.insert(0, '/opt/trn_rl_repo')

import ml_dtypes
import numpy as np

import concourse.bass as bass
import concourse.tile as tile
from concourse import bacc, mybir, library_config
from concourse.bass_utils import run_bass_kernel_spmd

N_CORES = 8
B, S, D = 4, 2048, 1024
H = 16
DH = 64                 # head dim
HC = 8                  # heads per core
C = HC * DH             # per-core projection width = 512
F32 = mybir.dt.float32
BF16 = mybir.dt.bfloat16
FP8 = mybir.dt.float8e4

NKT = S // 128          # 16 s-tiles of 128
NJ = NKT // 2           # 8 kt-pairs
NM = C // 128           # 4 head pairs
NDK = D // 128          # 8 contraction tiles for projections
SCALE = 1.0 / np.sqrt(DH)
EXP_BIAS = -4.0         # logit shift: exp(s/8 - 4), cancels in softmax
                        # (max scaled logit measured 8.29; e^4.3=73 < fp8 240)
VPAD = 80 if False else 65   # pad only needed for fp8 DoubleRow stride

PROJ_DT = BF16          # projection inputs (XT, W)
QK_DT = BF16            # qT/kT tiles (scores matmul inputs)
OUT_DT = BF16           # attn_outT + Wo (output projection inputs)
PV_FP8 = False          # fp8e4+DoubleRow PV: ~3.3e-2 err, too coarse


def prep(x, dt):
    return np.ascontiguousarray(x).astype(ml_dtypes.bfloat16)


def build():
    nc = bacc.Bacc("TRN2", target_bir_lowering=False, debug=False,
                   num_devices=N_CORES)
    XqT = nc.dram_tensor("XqT", [D, S], PROJ_DT, kind="ExternalInput").ap()
    XkT = nc.dram_tensor("XkT", [D, S], PROJ_DT, kind="ExternalInput").ap()
    XvT = nc.dram_tensor("XvT", [D, S], PROJ_DT, kind="ExternalInput").ap()
    Wq = nc.dram_tensor("Wq", [D, C], PROJ_DT, kind="ExternalInput").ap()
    Wk = nc.dram_tensor("Wk", [D, C], PROJ_DT, kind="ExternalInput").ap()
    Wv = nc.dram_tensor("Wv", [D, C], PROJ_DT, kind="ExternalInput").ap()
    Wo = nc.dram_tensor("Wo", [C, D], OUT_DT, kind="ExternalInput").ap()
    bq = nc.dram_tensor("bq", [C], F32, kind="ExternalInput").ap()
    bk = nc.dram_tensor("bk", [C], F32, kind="ExternalInput").ap()
    bv = nc.dram_tensor("bv", [C], F32, kind="ExternalInput").ap()
    OP = nc.dram_tensor("OP", [S, D], F32, kind="ExternalOutput").ap()

    with tile.TileContext(nc) as tc:
        _build_body(nc, tc, XqT, XkT, XvT, Wq, Wk, Wv, Wo, bq, bk, bv, OP)
    nc.compile()
    return nc


def _build_body(nc, tc, XqT, XkT, XvT, Wq, Wk, Wv, Wo, bq, bk, bv, OP):
    from contextlib import ExitStack
    with ExitStack() as stack:
        ep = stack.enter_context
        consts = ep(tc.tile_pool(name="consts", bufs=1))
        qkp = ep(tc.tile_pool(name="qk", bufs=2 * NM))
        vhp = ep(tc.tile_pool(name="vh", bufs=NJ))
        aop = ep(tc.tile_pool(name="aout", bufs=NM))
        wop = ep(tc.tile_pool(name="wo", bufs=NM))
        # 5 big X tiles alive (xk0,xk1,xq0,xv0,xv1); xq1 reuses the
        # xk0 buffer only after all kT chunks are emitted (qc0 end)
        xtp = ep(tc.tile_pool(name="xt", bufs=5))
        wp = ep(tc.tile_pool(name="w", bufs=3))
        ptp = ep(tc.tile_pool(name="pt", bufs=3))
        stg = ep(tc.tile_pool(name="stg", bufs=2))
        nrm = ep(tc.tile_pool(name="nrm", bufs=6))
        osg = ep(tc.tile_pool(name="osg", bufs=2))
        sp = ep(tc.tile_pool(name="sps", bufs=2, space="PSUM"))
        pvp = ep(tc.tile_pool(name="pv", bufs=2, space="PSUM"))
        scr = ep(tc.tile_pool(name="scr", bufs=2, space="PSUM"))

        nc.gpsimd.load_library(library_config.attn)

        # ---- constants ----
        neg2 = consts.tile([128, 1], F32)
        nc.vector.memset(neg2, EXP_BIAS)
        bias_t = consts.tile([128, 2 * NM], F32)
        for i, b_ in enumerate((bq, bk)):
            nc.gpsimd.dma_start(
                out=bias_t[:, i * NM:(i + 1) * NM],
                in_=b_.rearrange("(m p) -> p m", p=128))
        bvb = consts.tile([128, C], F32)
        nc.gpsimd.dma_start(
            out=bvb,
            in_=bass.AP(tensor=bv.tensor, offset=0, ap=[[0, 128], [1, C]]))

        wo_tiles = []
        for m in range(NM):
            w = wop.tile([128, D], OUT_DT, tag="wo", name=f"wo{m}")
            nc.gpsimd.dma_start(out=w, in_=Wo[m * 128:(m + 1) * 128, :])
            wo_tiles.append(w)

        # ---- persistent big tiles ----
        qT = [qkp.tile([128, S], QK_DT, tag="qk", name=f"qT{m}")
              for m in range(NM)]
        kT = [qkp.tile([128, S], QK_DT, tag="qk", name=f"kT{m}")
              for m in range(NM)]
        attn_outT = [aop.tile([128, S], OUT_DT, tag="aout", name=f"aoT{m}")
                     for m in range(NM)]
        PT_DT = FP8 if PV_FP8 else BF16
        # vh2[j]: [128(k within tile), head, po(kt pair sub), VPAD]
        vh2 = [vhp.tile([128, HC, 2, VPAD], PT_DT, tag="vh", name=f"vh{j}")
               for j in range(NJ)]
        for j in range(NJ):
            nc.vector.memset(vh2[j][:, :, :, DH:DH + 1], 1.0)

        # ---- projection helpers: single-DMA big tiles ----
        # X half: [128, dk, 1024] one DMA; W: [128, dk, C] one DMA
        def load_x(XT, half, tag):
            xt = xtp.tile([128, NDK, S // 2], PROJ_DT, tag="xt",
                          name=f"x{tag}{half}")
            nc.sync.dma_start(
                out=xt,
                in_=XT[:, half * (S // 2):(half + 1) * (S // 2)].rearrange(
                    "(a p) s -> p a s", p=128))
            return xt

        def load_w(W, tag):
            w = wp.tile([128, NDK, C], PROJ_DT, tag="w", name=f"w{tag}")
            nc.sync.dma_start(
                out=w, in_=W.rearrange("(a p) c -> p a c", p=128))
            return w

        def load_w_m(W, tag, w=None, ms=range(NM)):
            # per-m column slices: the first chunk needs only m=0
            if w is None:
                w = wp.tile([128, NDK, C], PROJ_DT, tag="w", name=f"w{tag}")
            for m in ms:
                nc.sync.dma_start(
                    out=w[:, :, m * 128:(m + 1) * 128],
                    in_=W[:, m * 128:(m + 1) * 128].rearrange(
                        "(a p) c -> p a c", p=128))
            return w

        def qk_mms(ps, xt, w, m, sc, dks):
            for dk in dks:
                nc.tensor.matmul(
                    ps,
                    w[:, dk, m * 128:(m + 1) * 128],
                    xt[:, dk, sc * 512:(sc + 1) * 512],
                    start=(dk == 0), stop=(dk == NDK - 1))

        def qk_epi(ps, m, half, sc, bcol, outs):
            s0 = half * (S // 2) + sc * 512
            with nc.allow_low_precision(reason="proj epilogue"):
                nc.vector.tensor_scalar_add(
                    outs[m][:, s0:s0 + 512], ps,
                    bias_t[:, bcol + m:bcol + m + 1])

        def qk_piece(xt, w, m, half, sc, bcol, outs):
            """One [128,512] projection chunk: 8 acc-mm + DVE epilogue."""
            ps = scr.tile([128, 512], F32, tag="scr")
            qk_mms(ps, xt, w, m, sc, range(NDK))
            qk_epi(ps, m, half, sc, bcol, outs)

        def qk_split(xt, w, m, half, sc, bcol, outs, parts=2):
            """Split into sub-pieces; accumulation group pauses between."""
            box = {}
            step = NDK // parts

            def mk(i):
                def p():
                    if i == 0:
                        ps = scr.tile([128, 512], F32, tag="scr")
                        box['ps'] = ps
                    qk_mms(box['ps'], xt, w, m, sc,
                           range(i * step, (i + 1) * step))
                    if i == parts - 1:
                        qk_epi(box['ps'], m, half, sc, bcol, outs)
                return p

            return [mk(i) for i in range(parts)]

        def v_piece(xt, w, st):
            """v-hat for s-tile st -> vh2[st//2][:, :, st%2, 0:64] (+bias)."""
            ps = scr.tile([128, C], F32, tag="scr")
            for dk in range(NDK):
                nc.tensor.matmul(
                    ps,
                    xt[:, dk, (st % 8) * 128:(st % 8 + 1) * 128],
                    w[:, dk, :],
                    start=(dk == 0), stop=(dk == NDK - 1))
            dst = vh2[st // 2][:, :, st % 2, 0:DH]
            with nc.allow_low_precision(reason="v epilogue"):
                nc.vector.tensor_add(
                    dst,
                    ps.rearrange("p (h d) -> p h d", h=HC),
                    bvb.rearrange("p (h d) -> p h d", h=HC))

        o3 = {}

        def o3_acc(i):
            # accumulators live in dead X-tile buffers (xv0/xq0 slots)
            if 'a' not in o3:
                t1 = xtp.tile([128, 4, 512], F32, tag="xt", name="o3a")
                t2 = xtp.tile([128, 4, 512], F32, tag="xt", name="o3b")
                o3['a'] = (t1, t2)
            t = o3['a'][i // 4]
            return t[:, i % 4, :]

        def outproj3_piece(st, oc, hp):
            """qc3: per-hp mm + SBUF accumulate; DMA on the last hp."""
            ps = scr.tile([128, 512], F32, tag="scr")
            nc.tensor.matmul(
                ps,
                attn_outT[hp][:, st * 128:(st + 1) * 128],
                wo_tiles[hp][:, oc * 512:(oc + 1) * 512],
                start=True, stop=True)
            oa = o3_acc((st - 12) * 2 + oc)
            if hp == 0:
                nc.vector.tensor_copy(oa, ps)
            else:
                nc.vector.tensor_add(oa, oa, ps)
            if hp == NM - 1:
                nc.sync.dma_start(
                    out=OP[st * 128:(st + 1) * 128,
                           oc * 512:(oc + 1) * 512],
                    in_=oa)

        def outproj_piece(st, oc):
            """OP[st,oc] chunk: 4 hp-accumulated mm + stage + DMA."""
            ps = scr.tile([128, 512], F32, tag="scr")
            for hp in range(NM):
                nc.tensor.matmul(
                    ps,
                    attn_outT[hp][:, st * 128:(st + 1) * 128],
                    wo_tiles[hp][:, oc * 512:(oc + 1) * 512],
                    start=(hp == 0), stop=(hp == NM - 1))
            oa = osg.tile([128, 512], F32, tag="osg")
            nc.vector.tensor_copy(oa, ps)
            nc.sync.dma_start(
                out=OP[st * 128:(st + 1) * 128, oc * 512:(oc + 1) * 512],
                in_=oa)

        # ---------------- prefix (serial head, ACT idle) ----------------
        # DMA order matches the PE's FIFO consumption order exactly:
        # kchunk(0,h0,*) -> qchunk(0) -> v0-5 -> kT half1 + v6-15 in (0,0)
        xk = [load_x(XkT, 0, "k"), None]
        wk = load_w_m(Wk, "k", ms=[0, 1])
        xv = [load_x(XvT, 0, "v"), None]
        wv = load_w(Wv, "v")
        xq = [load_x(XqT, 0, "q"), None]
        wq = load_w_m(Wq, "q", ms=[0, 1])
        xk[1] = load_x(XkT, 1, "k")
        xv[1] = load_x(XvT, 1, "v")
        load_w_m(Wk, "k", w=wk, ms=[2, 3])
        load_w_m(Wq, "q", w=wq, ms=[2, 3])

        def kchunk(m, half, sc):
            qk_piece(xk[half], wk, m, half, sc, NM, kT)

        def kchunk_split(m, half, sc, parts=2):
            return qk_split(xk[half], wk, m, half, sc, NM, kT, parts)

        def qchunk(m, half, sc):
            qk_piece(xq[half], wq, m, half, sc, 0, qT)

        def qchunk_split(m, half, sc, parts=2):
            return qk_split(xq[half], wq, m, half, sc, 0, qT, parts)

        # minimal prefix in PE-FIFO = DMA-arrival order: kT[0] half0
        # (+ kT[1]'s first chunk in the v-DMA wait window), v0-5, then
        # qT[0]/qT[1] qc0 -- PV(0) has v0 ready when exp starts
        kchunk(0, 0, 0)
        kchunk(0, 0, 1)
        kchunk(1, 0, 0)
        for st in range(6):
            v_piece(xv[0], wv, st)
        qchunk(0, 0, 0)
        qchunk(1, 0, 0)

        # ---------------- attention ----------------
        # slot_plan: mandatory pieces at exact kt slots (deps: a chunk
        # must be emitted before the instruction that reads it).
        # deferred: paced deque of (weight, fn) for slack work.
        deferred = deque()
        state = {'cooldown': 0}

        def fire():
            if state['cooldown'] > 0:
                state['cooldown'] -= 1
                return
            if deferred:
                w, fn = deferred.popleft()
                fn()
                state['cooldown'] = w - 1

        def plan_for(qc, hp):
            # a chunk must be EMITTED before the instruction reading it;
            # late slots of the previous block prefetch the next block.
            # post-plan pieces fire between exp and PV (so scores/exp of
            # the slot are not queued behind them on the PE).
            plan = {}
            post = {}

            def put(kt, fn):
                plan.setdefault(kt, []).append(fn)

            if qc == 0 and hp == 0:
                # v6-15: a slot ahead of the PV that reads them
                for st in range(6, NKT):
                    post.setdefault(st - 1, []).append(
                        lambda st=st: v_piece(xv[st // 8], wv, st))
                put(3, lambda: kchunk(0, 1, 0))
                put(4, lambda: kchunk(0, 1, 1))
            if qc == 0 and hp in (1, 2):
                # prefetch next hp's first chunks
                put(13, lambda hp=hp: kchunk(hp + 1, 0, 0))
                put(15, lambda hp=hp: qchunk(hp + 1, 0, 0))
            if qc == 0 and hp > 0:
                for s0, half, sc in ((0, 0, 1), (4, 1, 0), (8, 1, 1)):
                    for i, p in enumerate(kchunk_split(hp, half, sc, 4)):
                        put(s0 + i, p)
            if qc == 0 and hp == 3:
                for i, p in enumerate(qchunk_split(0, 0, 1, 4)):
                    put(12 + i, p)
            if qc == 1:
                # prefetch next block's qc1 q chunk, split across slots
                if hp < 3:
                    for i, p in enumerate(qchunk_split(hp + 1, 0, 1, 4)):
                        put(12 + i, p)
            return plan, post

        for qc in range(4):
            q0 = qc * 512
            for hp in range(NM):
                slot_plan, slot_post = plan_for(qc, hp)
                pvA = pvp.tile([DH + 1, 512], F32, tag="pv",
                               name=f"pvA{qc}_{hp}")
                pvB = pvp.tile([DH + 1, 512], F32, tag="pv",
                               name=f"pvB{qc}_{hp}")
                pend_pv = deque()
                for kt in range(NKT):
                    for fn in slot_plan.get(kt, ()):
                        fn()
                    if kt >= 2:
                        # keep block-start slots clear: the first scores
                        # must not queue behind deferred PE pieces
                        fire()
                    sps = sp.tile([128, 1024], F32, tag="sps")
                    for hh in range(2):
                        dlo = hh * DH
                        nc.tensor.matmul(
                            sps[:, hh * 512:(hh + 1) * 512],
                            kT[hp][dlo:dlo + DH, kt * 128:(kt + 1) * 128],
                            qT[hp][dlo:dlo + DH, q0:q0 + 512],
                            start=True, stop=True)
                    pt_k = ptp.tile([128, 2, 512], PT_DT, tag="pt")
                    with nc.allow_low_precision(reason="exp out"):
                        nc.scalar.activation(
                            out=pt_k.rearrange("p a b -> p (a b)"),
                            in_=sps,
                            func=mybir.ActivationFunctionType.Exp,
                            bias=neg2, scale=float(SCALE))
                    for fn in slot_post.get(kt, ()):
                        fn()
                    # PV(kt) emits two slots later: the first PVs of a
                    # block then don't wait on the previous block's
                    # stage/recip draining the pv psum buffers
                    if len(pend_pv) >= 2:
                        pend_pv.popleft()()

                    def mk_pv(kt=kt, pt_k=pt_k):
                        j, po = kt // 2, kt % 2
                        nc.tensor.matmul(
                            pvA, vh2[j][:, 2 * hp, po, 0:DH + 1],
                            pt_k[:, 0, :],
                            start=(kt == 0), stop=(kt == NKT - 1))
                        nc.tensor.matmul(
                            pvB, vh2[j][:, 2 * hp + 1, po, 0:DH + 1],
                            pt_k[:, 1, :],
                            start=(kt == 0), stop=(kt == NKT - 1))
                    pend_pv.append(mk_pv)
                while pend_pv:
                    pend_pv.popleft()()

                # ---- immediate tail: free pv psum fast ----
                stA = stg.tile([DH, 512], BF16, tag="stg")
                stB = stg.tile([DH, 512], BF16, tag="stg")
                rzA = nrm.tile([1, 512], BF16, tag="rz")
                rzB = nrm.tile([1, 512], BF16, tag="rz")
                with nc.allow_low_precision(reason="stage"):
                    nc.vector.tensor_copy(stA, pvA[0:DH, :])
                    nc.vector.tensor_copy(stB, pvB[0:DH, :])
                with nc.allow_low_precision(reason="recipZ bf16"):
                    nc.vector.reciprocal(out=rzA, in_=pvA[DH:DH + 1, :])
                    nc.vector.reciprocal(out=rzB, in_=pvB[DH:DH + 1, :])

                def mk_tail(hp=hp, q0=q0, stA=stA, stB=stB,
                            rzA=rzA, rzB=rzB):
                    bcA = nrm.tile([DH, 512], BF16, tag="bc")
                    bcB = nrm.tile([DH, 512], BF16, tag="bc")

                    def t0():
                        nc.gpsimd.partition_broadcast(bcA, rzA)

                    def t1():
                        nc.gpsimd.partition_broadcast(bcB, rzB)

                    def t2():
                        with nc.allow_low_precision(reason="attn out"):
                            nc.vector.tensor_mul(
                                attn_outT[hp][0:DH, q0:q0 + 512], stA, bcA)

                    def t3():
                        with nc.allow_low_precision(reason="attn out"):
                            nc.vector.tensor_mul(
                                attn_outT[hp][DH:128, q0:q0 + 512], stB, bcB)

                    return [(1, t0), (1, t1), (1, t2), (1, t3)]

                deferred.extend(mk_tail())

                if qc == 3:
                    deferred.extend(
                        (1, (lambda st=st, oc=oc, hp=hp:
                             outproj3_piece(st, oc, hp)))
                        for st in range(12, 16) for oc in range(2))

                if qc == 0 and hp == 3:
                    # q half1: DMA now (reuses the xk0 buffer; all kT
                    # half0 chunks already emitted). sc=0 chunks (read
                    # by qc2) fire in qc1; sc=1 (read by qc3) in qc2.
                    xq[1] = load_x(XqT, 1, "q")
                    for m in range(NM):
                        deferred.extend(
                            (1, p) for p in qchunk_split(m, 1, 0, 4))
                if qc == 1 and hp == 3:
                    for m in range(NM):
                        deferred.extend(
                            (1, p) for p in qchunk_split(m, 1, 1, 4))
            if qc < 3:
                # hp-accumulated outproj fires in the next qc's slots
                deferred.extend(
                    (2, (lambda st=st, oc=oc: outproj_piece(st, oc)))
                    for st in range(qc * 4, qc * 4 + 4) for oc in range(2))

        while deferred:
            deferred.popleft()[1]()


_NC_CACHE = None
_last_in_maps = None


def _get_nc():
    global _NC_CACHE
    if _NC_CACHE is None:
        _NC_CACHE = build()
    return _NC_CACHE


def kernel(Q, K, V, W_Q, b_Q, W_K, b_K, W_V, b_V, W_O, b_O):
    global _last_in_maps
    Q = np.asarray(Q, dtype=np.float32)
    K = np.asarray(K, dtype=np.float32)
    V = np.asarray(V, dtype=np.float32)
    nc = _get_nc()

    XqTs = [prep(Q[b].T, PROJ_DT) for b in range(B)]
    XkTs = [prep(K[b].T, PROJ_DT) for b in range(B)]
    XvTs = [prep(V[b].T, PROJ_DT) for b in range(B)]
    Wqs = [prep(np.asarray(W_Q)[:, hg * C:(hg + 1) * C], PROJ_DT)
           for hg in range(2)]
    Wks = [prep(np.asarray(W_K)[:, hg * C:(hg + 1) * C], PROJ_DT)
           for hg in range(2)]
    Wvs = [prep(np.asarray(W_V)[:, hg * C:(hg + 1) * C], PROJ_DT)
           for hg in range(2)]
    Wos = [prep(np.asarray(W_O)[hg * C:(hg + 1) * C, :], OUT_DT)
           for hg in range(2)]
    bqs = [np.ascontiguousarray(np.asarray(b_Q, np.float32)[hg * C:(hg + 1) * C])
           for hg in range(2)]
    bks = [np.ascontiguousarray(np.asarray(b_K, np.float32)[hg * C:(hg + 1) * C])
           for hg in range(2)]
    bvs = [np.ascontiguousarray(np.asarray(b_V, np.float32)[hg * C:(hg + 1) * C])
           for hg in range(2)]

    in_maps = []
    for c in range(N_CORES):
        b, hg = c // 2, c % 2
        in_maps.append({
            "XqT": XqTs[b], "XkT": XkTs[b], "XvT": XvTs[b],
            "Wq": Wqs[hg], "Wk": Wks[hg], "Wv": Wvs[hg], "Wo": Wos[hg],
            "bq": bqs[hg], "bk": bks[hg], "bv": bvs[hg],
        })
    _last_in_maps = in_maps
    res = run_bass_kernel_spmd(nc, in_maps, list(range(N_CORES)))
    out = np.empty((B, S, D), dtype=np.float32)
    bO = np.asarray(b_O, dtype=np.float32)
    for b in range(B):
        out[b] = res.results[2 * b]["OP"] + res.results[2 * b + 1]["OP"] + bO
    return out


# revision 47
# speedup vs baseline: 1.0074x; 1.0061x over previous
"""Multi-head attention (B=4, S=2048, D=1024, H=16) on 8 trn2 NeuronCores.

Sharding: core c -> (batch b = c//2, head-group hg = c%2 of 8 heads).
Each core computes q/k/v projections for its 8 heads, attention, and a
partial output projection. Host sums the two partials per batch + b_O.

Design: the ACT engine's exp stream is the roofline (~1.15us per
[128,1024] score tile); everything else is scheduled to hide under it.
  - ACT runs ONLY the 256 exp instructions (scale=1/8, bias=-4:
    softmax is shift-invariant; keeps exp small and overflow-safe).
  - v-hat is [128, head, po, 65] bf16 with a ones column so softmax Z
    falls out of the PV matmul (psum row 64). PV emission is delayed
    two slots so block boundaries never stall on stage/recip drains.
  - scores matmuls: head pair at base partitions 0/64 -> row-tile
    concurrency on the PE array (tile_position auto-derived).
  - loops are qc-outer / hp-inner so the output projection can
    PSUM-accumulate across all 4 head pairs per piece; pieces fire in
    the next qc block's kt slots (weighted pacing).
  - recipZ broadcast runs on the idle Pool engine (partition_broadcast)
  - projection bias epilogues run on DVE (tensor_scalar_add), not ACT.
  - projections are JIT slot-planned pieces inside the attention
    stream: the serial prefix is only kT[0] + qT[0]/qc0 + v half.
    Remaining kT[hp]/qT[hp] chunks fire just before the block that
    reads them; q half1 fires paced during qc1; v st8-15 inside qc0/hp0.
  - qc3's output projection switches to per-hp SBUF accumulation so
    pieces drain during the last blocks instead of a serial tail.
"""
import sys
from collections import deque

if '/opt/trn_rl_repo' not in sys.path:
    sys.path.insert(0, '/opt/trn_rl_repo')

import ml_dtypes
import numpy as np

import concourse.bass as bass
import concourse.tile as tile
from concourse import bacc, mybir, library_config
from concourse.bass_utils import run_bass_kernel_spmd

N_CORES = 8
B, S, D = 4, 2048, 1024
H = 16
DH = 64                 # head dim
HC = 8                  # heads per core
C = HC * DH             # per-core projection width = 512
F32 = mybir.dt.float32
BF16 = mybir.dt.bfloat16
FP8 = mybir.dt.float8e4

NKT = S // 128          # 16 s-tiles of 128
NJ = NKT // 2           # 8 kt-pairs
NM = C // 128           # 4 head pairs
NDK = D // 128          # 8 contraction tiles for projections
SCALE = 1.0 / np.sqrt(DH)
EXP_BIAS = -4.0         # logit shift: exp(s/8 - 4), cancels in softmax
                        # (max scaled logit measured 8.29; e^4.3=73 < fp8 240)
VPAD = 80 if False else 65   # pad only needed for fp8 DoubleRow stride

PROJ_DT = BF16          # projection inputs (XT, W)
QK_DT = BF16            # qT/kT tiles (scores matmul inputs)
OUT_DT = BF16           # attn_outT + Wo (output projection inputs)
PV_FP8 = False          # fp8e4+DoubleRow PV: ~3.3e-2 err, too coarse


def prep(x, dt):
    return np.ascontiguousarray(x).astype(ml_dtypes.bfloat16)


def build():
    nc = bacc.Bacc("TRN2", target_bir_lowering=False, debug=False,
                   num_devices=N_CORES)
    XqT = nc.dram_tensor("XqT", [D, S], PROJ_DT, kind="ExternalInput").ap()
    XkT = nc.dram_tensor("XkT", [D, S], PROJ_DT, kind="ExternalInput").ap()
    XvT = nc.dram_tensor("XvT", [D, S], PROJ_DT, kind="ExternalInput").ap()
    Wq = nc.dram_tensor("Wq", [D, C], PROJ_DT, kind="ExternalInput").ap()
    Wk = nc.dram_tensor("Wk", [D, C], PROJ_DT, kind="ExternalInput").ap()
    Wv = nc.dram_tensor("Wv", [D, C], PROJ_DT, kind="ExternalInput").ap()
    Wo = nc.dram_tensor("Wo", [C, D], OUT_DT, kind="ExternalInput").ap()
    bq = nc.dram_tensor("bq", [C], F32, kind="ExternalInput").ap()
    bk = nc.dram_tensor("bk", [C], F32, kind="ExternalInput").ap()
    bv = nc.dram_tensor("bv", [C], F32, kind="ExternalInput").ap()
    OP = nc.dram_tensor("OP", [S, D], F32, kind="ExternalOutput").ap()

    with tile.TileContext(nc) as tc:
        _build_body(nc, tc, XqT, XkT, XvT, Wq, Wk, Wv, Wo, bq, bk, bv, OP)
    nc.compile()
    return nc


def _build_body(nc, tc, XqT, XkT, XvT, Wq, Wk, Wv, Wo, bq, bk, bv, OP):
    from contextlib import ExitStack
    with ExitStack() as stack:
        ep = stack.enter_context
        consts = ep(tc.tile_pool(name="consts", bufs=1))
        qkp = ep(tc.tile_pool(name="qk", bufs=2 * NM))
        vhp = ep(tc.tile_pool(name="vh", bufs=NJ))
        aop = ep(tc.tile_pool(name="aout", bufs=NM))
        wop = ep(tc.tile_pool(name="wo", bufs=NM))
        # 5 big X tiles alive (xk0,xk1,xq0,xv0,xv1); xq1 reuses the
        # xk0 buffer only after all kT chunks are emitted (qc0 end)
        xtp = ep(tc.tile_pool(name="xt", bufs=5))
        wp = ep(tc.tile_pool(name="w", bufs=3))
        ptp = ep(tc.tile_pool(name="pt", bufs=4))
        stg = ep(tc.tile_pool(name="stg", bufs=2))
        nrm = ep(tc.tile_pool(name="nrm", bufs=6))
        osg = ep(tc.tile_pool(name="osg", bufs=2))
        sp = ep(tc.tile_pool(name="sps", bufs=2, space="PSUM"))
        pvp = ep(tc.tile_pool(name="pv", bufs=2, space="PSUM"))
        scr = ep(tc.tile_pool(name="scr", bufs=2, space="PSUM"))

        nc.gpsimd.load_library(library_config.attn)

        # ---- constants ----
        neg2 = consts.tile([128, 1], F32)
        nc.vector.memset(neg2, EXP_BIAS)
        bias_t = consts.tile([128, 2 * NM], F32)
        for i, b_ in enumerate((bq, bk)):
            nc.gpsimd.dma_start(
                out=bias_t[:, i * NM:(i + 1) * NM],
                in_=b_.rearrange("(m p) -> p m", p=128))
        bvb = consts.tile([128, C], F32)
        nc.gpsimd.dma_start(
            out=bvb,
            in_=bass.AP(tensor=bv.tensor, offset=0, ap=[[0, 128], [1, C]]))

        wo_tiles = []
        for m in range(NM):
            w = wop.tile([128, D], OUT_DT, tag="wo", name=f"wo{m}")
            nc.gpsimd.dma_start(out=w, in_=Wo[m * 128:(m + 1) * 128, :])
            wo_tiles.append(w)

        # ---- persistent big tiles ----
        qT = [qkp.tile([128, S], QK_DT, tag="qk", name=f"qT{m}")
              for m in range(NM)]
        kT = [qkp.tile([128, S], QK_DT, tag="qk", name=f"kT{m}")
              for m in range(NM)]
        attn_outT = [aop.tile([128, S], OUT_DT, tag="aout", name=f"aoT{m}")
                     for m in range(NM)]
        PT_DT = FP8 if PV_FP8 else BF16
        # vh2[j]: [128(k within tile), head, po(kt pair sub), VPAD]
        vh2 = [vhp.tile([128, HC, 2, VPAD], PT_DT, tag="vh", name=f"vh{j}")
               for j in range(NJ)]
        for j in range(NJ):
            nc.vector.memset(vh2[j][:, :, :, DH:DH + 1], 1.0)

        # ---- projection helpers: single-DMA big tiles ----
        # X half: [128, dk, 1024] one DMA; W: [128, dk, C] one DMA
        def load_x(XT, half, tag):
            xt = xtp.tile([128, NDK, S // 2], PROJ_DT, tag="xt",
                          name=f"x{tag}{half}")
            nc.sync.dma_start(
                out=xt,
                in_=XT[:, half * (S // 2):(half + 1) * (S // 2)].rearrange(
                    "(a p) s -> p a s", p=128))
            return xt

        def load_w(W, tag):
            w = wp.tile([128, NDK, C], PROJ_DT, tag="w", name=f"w{tag}")
            nc.sync.dma_start(
                out=w, in_=W.rearrange("(a p) c -> p a c", p=128))
            return w

        def load_w_m(W, tag, w=None, ms=range(NM)):
            # per-m column slices: the first chunk needs only m=0
            if w is None:
                w = wp.tile([128, NDK, C], PROJ_DT, tag="w", name=f"w{tag}")
            for m in ms:
                nc.sync.dma_start(
                    out=w[:, :, m * 128:(m + 1) * 128],
                    in_=W[:, m * 128:(m + 1) * 128].rearrange(
                        "(a p) c -> p a c", p=128))
            return w

        def qk_mms(ps, xt, w, m, sc, dks):
            for dk in dks:
                nc.tensor.matmul(
                    ps,
                    w[:, dk, m * 128:(m + 1) * 128],
                    xt[:, dk, sc * 512:(sc + 1) * 512],
                    start=(dk == 0), stop=(dk == NDK - 1))

        def qk_epi(ps, m, half, sc, bcol, outs):
            s0 = half * (S // 2) + sc * 512
            with nc.allow_low_precision(reason="proj epilogue"):
                nc.vector.tensor_scalar_add(
                    outs[m][:, s0:s0 + 512], ps,
                    bias_t[:, bcol + m:bcol + m + 1])

        def qk_piece(xt, w, m, half, sc, bcol, outs):
            """One [128,512] projection chunk: 8 acc-mm + DVE epilogue."""
            ps = scr.tile([128, 512], F32, tag="scr")
            qk_mms(ps, xt, w, m, sc, range(NDK))
            qk_epi(ps, m, half, sc, bcol, outs)

        def qk_split(xt, w, m, half, sc, bcol, outs, parts=2):
            """Split into sub-pieces; accumulation group pauses between."""
            box = {}
            step = NDK // parts

            def mk(i):
                def p():
                    if i == 0:
                        ps = scr.tile([128, 512], F32, tag="scr")
                        box['ps'] = ps
                    qk_mms(box['ps'], xt, w, m, sc,
                           range(i * step, (i + 1) * step))
                    if i == parts - 1:
                        qk_epi(box['ps'], m, half, sc, bcol, outs)
                return p

            return [mk(i) for i in range(parts)]

        def v_piece(xt, w, st):
            """v-hat for s-tile st -> vh2[st//2][:, :, st%2, 0:64] (+bias)."""
            ps = scr.tile([128, C], F32, tag="scr")
            for dk in range(NDK):
                nc.tensor.matmul(
                    ps,
                    xt[:, dk, (st % 8) * 128:(st % 8 + 1) * 128],
                    w[:, dk, :],
                    start=(dk == 0), stop=(dk == NDK - 1))
            dst = vh2[st // 2][:, :, st % 2, 0:DH]
            with nc.allow_low_precision(reason="v epilogue"):
                nc.vector.tensor_add(
                    dst,
                    ps.rearrange("p (h d) -> p h d", h=HC),
                    bvb.rearrange("p (h d) -> p h d", h=HC))

        o3 = {}

        def o3_acc(i):
            # accumulators live in dead X-tile buffers (xv0/xq0 slots)
            if 'a' not in o3:
                t1 = xtp.tile([128, 4, 512], F32, tag="xt", name="o3a")
                t2 = xtp.tile([128, 4, 512], F32, tag="xt", name="o3b")
                o3['a'] = (t1, t2)
            t = o3['a'][i // 4]
            return t[:, i % 4, :]

        def outproj3_piece(st, oc, hp):
            """qc3: per-hp mm + SBUF accumulate; DMA on the last hp."""
            ps = scr.tile([128, 512], F32, tag="scr")
            nc.tensor.matmul(
                ps,
                attn_outT[hp][:, st * 128:(st + 1) * 128],
                wo_tiles[hp][:, oc * 512:(oc + 1) * 512],
                start=True, stop=True)
            oa = o3_acc((st - 12) * 2 + oc)
            if hp == 0:
                nc.vector.tensor_copy(oa, ps)
            else:
                nc.vector.tensor_add(oa, oa, ps)
            if hp == NM - 1:
                nc.sync.dma_start(
                    out=OP[st * 128:(st + 1) * 128,
                           oc * 512:(oc + 1) * 512],
                    in_=oa)

        def outproj_piece(st, oc):
            """OP[st,oc] chunk: 4 hp-accumulated mm + stage + DMA."""
            ps = scr.tile([128, 512], F32, tag="scr")
            for hp in range(NM):
                nc.tensor.matmul(
                    ps,
                    attn_outT[hp][:, st * 128:(st + 1) * 128],
                    wo_tiles[hp][:, oc * 512:(oc + 1) * 512],
                    start=(hp == 0), stop=(hp == NM - 1))
            oa = osg.tile([128, 512], F32, tag="osg")
            nc.vector.tensor_copy(oa, ps)
            nc.sync.dma_start(
                out=OP[st * 128:(st + 1) * 128, oc * 512:(oc + 1) * 512],
                in_=oa)

        # ---------------- prefix (serial head, ACT idle) ----------------
        # DMA order matches the PE's FIFO consumption order exactly:
        # kchunk(0,h0,*) -> qchunk(0) -> v0-5 -> kT half1 + v6-15 in (0,0)
        xk = [load_x(XkT, 0, "k"), None]
        wk = load_w_m(Wk, "k", ms=[0, 1])
        xv = [load_x(XvT, 0, "v"), None]
        wv = load_w(Wv, "v")
        xq = [load_x(XqT, 0, "q"), None]
        wq = load_w_m(Wq, "q", ms=[0, 1])
        xk[1] = load_x(XkT, 1, "k")
        xv[1] = load_x(XvT, 1, "v")
        load_w_m(Wk, "k", w=wk, ms=[2, 3])
        load_w_m(Wq, "q", w=wq, ms=[2, 3])

        def kchunk(m, half, sc):
            qk_piece(xk[half], wk, m, half, sc, NM, kT)

        def kchunk_split(m, half, sc, parts=2):
            return qk_split(xk[half], wk, m, half, sc, NM, kT, parts)

        def qchunk(m, half, sc):
            qk_piece(xq[half], wq, m, half, sc, 0, qT)

        def qchunk_split(m, half, sc, parts=2):
            return qk_split(xq[half], wq, m, half, sc, 0, qT, parts)

        # minimal prefix in PE-FIFO = DMA-arrival order: kT[0] half0
        # (+ kT[1]'s first chunk in the v-DMA wait window), v0-5, then
        # qT[0]/qT[1] qc0 -- PV(0) has v0 ready when exp starts
        kchunk(0, 0, 0)
        kchunk(0, 0, 1)
        kchunk(1, 0, 0)
        for st in range(6):
            v_piece(xv[0], wv, st)
        qchunk(0, 0, 0)
        qchunk(1, 0, 0)

        # ---------------- attention ----------------
        # slot_plan: mandatory pieces at exact kt slots (deps: a chunk
        # must be emitted before the instruction that reads it).
        # deferred: paced deque of (weight, fn) for slack work.
        deferred = deque()
        state = {'cooldown': 0}

        def fire():
            if state['cooldown'] > 0:
                state['cooldown'] -= 1
                return
            if deferred:
                w, fn = deferred.popleft()
                fn()
                state['cooldown'] = w - 1

        def plan_for(qc, hp):
            # a chunk must be EMITTED before the instruction reading it;
            # late slots of the previous block prefetch the next block.
            # post-plan pieces fire between exp and PV (so scores/exp of
            # the slot are not queued behind them on the PE).
            plan = {}
            post = {}

            def put(kt, fn):
                plan.setdefault(kt, []).append(fn)

            if qc == 0 and hp == 0:
                # v6-15: a slot ahead of the PV that reads them
                for st in range(6, NKT):
                    post.setdefault(st - 1, []).append(
                        lambda st=st: v_piece(xv[st // 8], wv, st))
                put(3, lambda: kchunk(0, 1, 0))
                put(4, lambda: kchunk(0, 1, 1))
            if qc == 0 and hp in (1, 2):
                # prefetch next hp's first chunks
                put(13, lambda hp=hp: kchunk(hp + 1, 0, 0))
                put(15, lambda hp=hp: qchunk(hp + 1, 0, 0))
            if qc == 0 and hp > 0:
                for s0, half, sc in ((0, 0, 1), (4, 1, 0), (8, 1, 1)):
                    for i, p in enumerate(kchunk_split(hp, half, sc, 4)):
                        put(s0 + i, p)
            if qc == 0 and hp == 3:
                for i, p in enumerate(qchunk_split(0, 0, 1, 4)):
                    put(12 + i, p)
            if qc == 1:
                # prefetch next block's qc1 q chunk, split across slots
                if hp < 3:
                    for i, p in enumerate(qchunk_split(hp + 1, 0, 1, 4)):
                        put(12 + i, p)
            return plan, post

        for qc in range(4):
            q0 = qc * 512
            for hp in range(NM):
                slot_plan, slot_post = plan_for(qc, hp)
                pvA = pvp.tile([DH + 1, 512], F32, tag="pv",
                               name=f"pvA{qc}_{hp}")
                pvB = pvp.tile([DH + 1, 512], F32, tag="pv",
                               name=f"pvB{qc}_{hp}")
                pend_pv = deque()
                for kt in range(NKT):
                    for fn in slot_plan.get(kt, ()):
                        fn()
                    if kt >= 2:
                        # keep block-start slots clear: the first scores
                        # must not queue behind deferred PE pieces
                        fire()
                    sps = sp.tile([128, 1024], F32, tag="sps")
                    for hh in range(2):
                        dlo = hh * DH
                        nc.tensor.matmul(
                            sps[:, hh * 512:(hh + 1) * 512],
                            kT[hp][dlo:dlo + DH, kt * 128:(kt + 1) * 128],
                            qT[hp][dlo:dlo + DH, q0:q0 + 512],
                            start=True, stop=True)
                    pt_k = ptp.tile([128, 2, 512], PT_DT, tag="pt")
                    with nc.allow_low_precision(reason="exp out"):
                        nc.scalar.activation(
                            out=pt_k.rearrange("p a b -> p (a b)"),
                            in_=sps,
                            func=mybir.ActivationFunctionType.Exp,
                            bias=neg2, scale=float(SCALE))
                    for fn in slot_post.get(kt, ()):
                        fn()
                    # PV(kt) emits two slots later: the first PVs of a
                    # block then don't wait on the previous block's
                    # stage/recip draining the pv psum buffers
                    if len(pend_pv) >= 2:
                        pend_pv.popleft()()

                    def mk_pv(kt=kt, pt_k=pt_k):
                        j, po = kt // 2, kt % 2
                        nc.tensor.matmul(
                            pvA, vh2[j][:, 2 * hp, po, 0:DH + 1],
                            pt_k[:, 0, :],
                            start=(kt == 0), stop=(kt == NKT - 1))
                        nc.tensor.matmul(
                            pvB, vh2[j][:, 2 * hp + 1, po, 0:DH + 1],
                            pt_k[:, 1, :],
                            start=(kt == 0), stop=(kt == NKT - 1))
                    pend_pv.append(mk_pv)
                while pend_pv:
                    pend_pv.popleft()()

                # ---- immediate tail: free pv psum fast ----
                stA = stg.tile([DH, 512], BF16, tag="stg")
                stB = stg.tile([DH, 512], BF16, tag="stg")
                rzA = nrm.tile([1, 512], BF16, tag="rz")
                rzB = nrm.tile([1, 512], BF16, tag="rz")
                with nc.allow_low_precision(reason="stage"):
                    nc.vector.tensor_copy(stA, pvA[0:DH, :])
                    nc.vector.tensor_copy(stB, pvB[0:DH, :])
                with nc.allow_low_precision(reason="recipZ bf16"):
                    nc.vector.reciprocal(out=rzA, in_=pvA[DH:DH + 1, :])
                    nc.vector.reciprocal(out=rzB, in_=pvB[DH:DH + 1, :])

                def mk_tail(hp=hp, q0=q0, stA=stA, stB=stB,
                            rzA=rzA, rzB=rzB):
                    bcA = nrm.tile([DH, 512], BF16, tag="bc")
                    bcB = nrm.tile([DH, 512], BF16, tag="bc")

                    def t0():
                        nc.gpsimd.partition_broadcast(bcA, rzA)

                    def t1():
                        nc.gpsimd.partition_broadcast(bcB, rzB)

                    def t2():
                        with nc.allow_low_precision(reason="attn out"):
                            nc.vector.tensor_mul(
                                attn_outT[hp][0:DH, q0:q0 + 512], stA, bcA)

                    def t3():
                        with nc.allow_low_precision(reason="attn out"):
                            nc.vector.tensor_mul(
                                attn_outT[hp][DH:128, q0:q0 + 512], stB, bcB)

                    return [(1, t0), (1, t1), (1, t2), (1, t3)]

                deferred.extend(mk_tail())

                if qc == 3:
                    deferred.extend(
                        (1, (lambda st=st, oc=oc, hp=hp:
                             outproj3_piece(st, oc, hp)))
                        for st in range(12, 16) for oc in range(2))

                if qc == 0 and hp == 3:
                    # q half1: DMA now (reuses the xk0 buffer; all kT
                    # half0 chunks already emitted). sc=0 chunks (read
                    # by qc2) fire in qc1; sc=1 (read by qc3) in qc2.
                    xq[1] = load_x(XqT, 1, "q")
                    for m in range(NM):
                        deferred.extend(
                            (1, p) for p in qchunk_split(m, 1, 0, 4))
                if qc == 1 and hp == 3:
                    for m in range(NM):
                        deferred.extend(
                            (1, p) for p in qchunk_split(m, 1, 1, 4))
            if qc < 3:
                # hp-accumulated outproj fires in the next qc's slots
                deferred.extend(
                    (2, (lambda st=st, oc=oc: outproj_piece(st, oc)))
                    for st in range(qc * 4, qc * 4 + 4) for oc in range(2))

        while deferred:
            deferred.popleft()[1]()


_NC_CACHE = None
_last_in_maps = None


def _get_nc():
    global _NC_CACHE
    if _NC_CACHE is None:
        _NC_CACHE = build()
    return _NC_CACHE


def kernel(Q, K, V, W_Q, b_Q, W_K, b_K, W_V, b_V, W_O, b_O):
    global _last_in_maps
    Q = np.asarray(Q, dtype=np.float32)
    K = np.asarray(K, dtype=np.float32)
    V = np.asarray(V, dtype=np.float32)
    nc = _get_nc()

    XqTs = [prep(Q[b].T, PROJ_DT) for b in range(B)]
    XkTs = [prep(K[b].T, PROJ_DT) for b in range(B)]
    XvTs = [prep(V[b].T, PROJ_DT) for b in range(B)]
    Wqs = [prep(np.asarray(W_Q)[:, hg * C:(hg + 1) * C], PROJ_DT)
           for hg in range(2)]
    Wks = [prep(np.asarray(W_K)[:, hg * C:(hg + 1) * C], PROJ_DT)
           for hg in range(2)]
    Wvs = [prep(np.asarray(W_V)[:, hg * C:(hg + 1) * C], PROJ_DT)
           for hg in range(2)]
    Wos = [prep(np.asarray(W_O)[hg * C:(hg + 1) * C, :], OUT_DT)
           for hg in range(2)]
    bqs = [np.ascontiguousarray(np.asarray(b_Q, np.float32)[hg * C:(hg + 1) * C])
           for hg in range(2)]
    bks = [np.ascontiguousarray(np.asarray(b_K, np.float32)[hg * C:(hg + 1) * C])
           for hg in range(2)]
    bvs = [np.ascontiguousarray(np.asarray(b_V, np.float32)[hg * C:(hg + 1) * C])
           for hg in range(2)]

    in_maps = []
    for c in range(N_CORES):
        b, hg = c // 2, c % 2
        in_maps.append({
            "XqT": XqTs[b], "XkT": XkTs[b], "XvT": XvTs[b],
            "Wq": Wqs[hg], "Wk": Wks[hg], "Wv": Wvs[hg], "Wo": Wos[hg],
            "bq": bqs[hg], "bk": bks[hg], "bv": bvs[hg],
        })
    _last_in_maps = in_maps
    res = run_bass_kernel_spmd(nc, in_maps, list(range(N_CORES)))
    out = np.empty((B, S, D), dtype=np.float32)
    bO = np.asarray(b_O, dtype=np.float32)
    for b in range(B):
        out[b] = res.results[2 * b]["OP"] + res.results[2 * b + 1]["OP"] + bO
    return out
